# revision 1
# baseline (speedup 1.0000x reference)
"""Causal self-attention (B=2, T=2048, C=1024, H=16, D=64) on 8 trn2 cores.

Sharding: core c = (batch b = c//4, head-group g = c%4 covering heads 4g..4g+3).
QKV projection is column-parallel over the core's 12 head-channels blocks,
attention is fully local per head, output projection is row-parallel with the
partial sums reduced on the host (plus bproj).

Device dataflow (per core, all fp32):
  xT (C,T) @ wqkvT (C,768) -> qkv psum in (t, ch) layout, bias via K=1 matmul
  RoPE on q,k applied at psum->sbuf eviction (host permuted W rows so each
  head's channels are [evens(32) | odds(32)])
  PE-transpose q,k -> (d, t) layout
  S^T = kT.T @ qT per (head, 128-k-chunk, 512-q-chunk), causal-skipped
  P^T = exp(S^T * 0.125) on ACT, triangular-mask add on band tiles before exp
  yT' = v'.T @ P^T accumulated over k-chunks, where v' has a ones column so
        row 64 of yT' is the softmax denominator
  normalize: reciprocal + DMA partition-broadcast + DVE multiply
  out_partial = Y.T @ wprojT -> (t, n), DMA to DRAM
"""

import math

import numpy as np

import concourse.bass as bass
import concourse.mybir as mybir
from concourse.tile import TileContext
from concourse.bass_utils import run_bass_kernel_spmd

B, T, C, H = 2, 2048, 1024, 16
D = C // H  # 64
ROPE_BASE = 10000.0
N_CORES = 8
HPC = H // 4  # heads per core = 4
CPC = HPC * D  # channels per core = 256
TT = T // 128  # 16 t-tiles
NJ = T // 512  # 4 q-chunks
F32 = mybir.dt.float32
F32R = mybir.dt.float32r


def _r(ap):
    return ap.bitcast(F32R)

def _split_sync_waits(nc, cap=1):
    """This walrus build rejects instructions carrying more than `cap` sem
    waits; hoist the excess onto same-engine NoOp carriers placed just
    before the instruction."""
    ctr = 0
    for fn in nc.m.functions:
        for blk in fn.blocks:
            out = []
            for inst in blk.instructions:
                si = inst.sync_info
                if si is not None and si.on_wait and len(si.on_wait) > cap:
                    waits = list(si.on_wait)
                    rest, keep = waits[:-cap], waits[-cap:]
                    for k in range(0, len(rest), cap):
                        ctr += 1
                        nop = mybir.InstNoOp(
                            name=f"waitsplit-{ctr}", ins=[], outs=[]
                        )
                        nop.engine = inst.engine
                        nop.sync_info = mybir.SyncInfo(
                            on_wait=rest[k : k + cap], on_update=[]
                        )
                        nc.register_instruction(nop)
                        out.append(nop)
                    si.on_wait[:] = keep
                out.append(inst)
            blk.instructions[:] = out


def build_nc(with_bias=True):
    nc = bass.Bass()

    xT = nc.dram_tensor("xT", [C, T], F32R, kind="ExternalInput")
    wqkvT = nc.dram_tensor("wqkvT", [C, 3 * CPC], F32R, kind="ExternalInput")
    bqkv_s = nc.dram_tensor("bqkv_s", [1, 3 * CPC], F32R, kind="ExternalInput")
    wprojT = nc.dram_tensor("wprojT", [CPC, C], F32R, kind="ExternalInput")
    cosr = nc.dram_tensor("cosr", [T, 256], F32, kind="ExternalInput")
    sinr = nc.dram_tensor("sinr", [T, 256], F32, kind="ExternalInput")
    trimask = nc.dram_tensor("trimask", [128, 256], F32, kind="ExternalInput")
    ident = nc.dram_tensor("ident", [128, 128], F32, kind="ExternalInput")
    onesr = nc.dram_tensor("onesr", [128, 128], F32R, kind="ExternalInput")
    out = nc.dram_tensor("out", [T, C], F32, kind="ExternalOutput")

    with TileContext(nc) as tc:
        with (
            tc.tile_pool(name="const", bufs=1) as cpool,
            tc.tile_pool(name="xin", bufs=4) as xpool,
            tc.tile_pool(name="qk", bufs=5) as qkpool,
            tc.tile_pool(name="pT", bufs=10) as ppool,
            tc.tile_pool(name="norm", bufs=4) as npool,
            tc.tile_pool(name="obuf", bufs=3) as opool,
            tc.tile_pool(name="persist", bufs=1) as perpool,
            tc.tile_pool(name="ps1", bufs=4, space="PSUM") as ps1,
            tc.tile_pool(name="psy", bufs=4, space="PSUM") as psy,
        ):
            # ---- weights / constants; chunked so the first matmuls can
            # start before the full load finishes ----
            wq_sb = cpool.tile([128, 8, 3 * CPC], F32R, tag="wq")

            def load_wq_chunk(kc):
                nc.sync.dma_start(
                    wq_sb[:, kc, :],
                    wqkvT.rearrange("(kc p) n -> p kc n", p=128)[:, kc, :],
                )

            xt_pre = {}

            def prefetch_xt(tt, split=False):
                xt = xpool.tile([128, 8, 128], F32R, tag="xt", name="xt")
                halves = ((0, 4), (4, 8)) if split else ((0, 8),)
                for lo, hi in halves:
                    nc.sync.dma_start(
                        xt[:, lo:hi, :],
                        xT.rearrange("(kc p) t -> p kc t", p=128)[
                            :, lo:hi, tt * 128 : (tt + 1) * 128
                        ],
                    )
                xt_pre[tt] = xt

            nc.sync.dma_start(
                wq_sb[:, 0, 0:512],
                wqkvT.rearrange("(kc p) n -> p kc n", p=128)[:, 0, 0:512],
            )
            prefetch_xt(0, split=True)
            nc.sync.dma_start(
                wq_sb[:, 0, 512:768],
                wqkvT.rearrange("(kc p) n -> p kc n", p=128)[:, 0, 512:768],
            )
            load_wq_chunk(1)
            prefetch_xt(1, split=True)
            for kc in range(2, 8):
                load_wq_chunk(kc)
            cos_sb = cpool.tile([128, TT, 256], F32, tag="cos")
            sin_sb = cpool.tile([128, TT, 256], F32, tag="sin")

            def load_cs_chunk(cc):
                sl = slice(cc * 2, cc * 2 + 2)
                nc.sync.dma_start(
                    cos_sb[:, sl, :],
                    cosr.rearrange("(tt p) f -> p tt f", p=128)[:, sl, :],
                )
                nc.sync.dma_start(
                    sin_sb[:, sl, :],
                    sinr.rearrange("(tt p) f -> p tt f", p=128)[:, sl, :],
                )

            prefetch_xt(2)
            load_cs_chunk(0)
            prefetch_xt(3)
            tri_sb = cpool.tile([128, 256], F32, tag="tri")
            nc.sync.dma_start(tri_sb[:], trimask[:, :])
            id_sb = cpool.tile([128, 128], F32, tag="id")
            nc.sync.dma_start(id_sb[:], ident[:, :])
            bq_sb = cpool.tile([1, 3 * CPC], F32R, tag="bq")
            nc.sync.dma_start(bq_sb[:], bqkv_s[:, :])
            onesr_sb = cpool.tile([128, 128], F32R, tag="onesr")
            nc.sync.dma_start(onesr_sb[:], onesr[:, :])
            onesf_sb = cpool.tile([1, 64], F32, tag="onesf")
            nc.vector.memset(onesf_sb[:], 1.0)
            wp_sb = cpool.tile([128, 2, C], F32R, tag="wp")  # loaded at tt=3

            # persistent activations
            v_sb = perpool.tile([128, TT, HPC * (D + 1)], F32R, tag="v")
            qkT_sb = perpool.tile([128, 4, TT, 128], F32R, tag="qkT")
            y_sb = [
                perpool.tile([128, T], F32R, tag=f"y{j}", name=f"y_sb{j}")
                for j in range(2)
            ]

            qs_pend = []

            def emit_transposes(pend):
                ptt, pqs = pend
                tp = ps1.tile([128, 512], F32, tag="ps1", name="tp")
                for s in range(4):  # q j0, q j1, k j0, k j1
                    nc.tensor.transpose(
                        tp[:, s * 128 : (s + 1) * 128],
                        pqs[:, s * 128 : (s + 1) * 128],
                        id_sb[:],
                    )
                nc.scalar.copy(qkT_sb[:, :, ptt, :], tp[:])

            def emit_phase1(tt):
                if tt in xt_pre:
                    xt = xt_pre.pop(tt)
                else:
                    prefetch_xt(tt)
                    xt = xt_pre.pop(tt)
                qk_ps = ps1.tile([128, 512], F32, tag="ps1", name="qk_ps")
                v_ps = ps1.tile([128, 256], F32, tag="ps1", name="v_ps")
                last = not with_bias
                for kc in range(8):
                    nc.tensor.matmul(
                        qk_ps[:],
                        lhsT=xt[:, kc, :],
                        rhs=wq_sb[:, kc, 0:512],
                        start=(kc == 0),
                        stop=(last and kc == 7),
                    )
                    nc.tensor.matmul(
                        v_ps[:],
                        lhsT=xt[:, kc, :],
                        rhs=wq_sb[:, kc, 512:768],
                        start=(kc == 0),
                        stop=(last and kc == 7),
                    )
                if with_bias:
                    nc.tensor.matmul(
                        qk_ps[:], lhsT=onesr_sb[0:1, :], rhs=bq_sb[:, 0:512],
                        start=False, stop=True,
                    )
                    nc.tensor.matmul(
                        v_ps[:], lhsT=onesr_sb[0:1, :], rhs=bq_sb[:, 512:768],
                        start=False, stop=True,
                    )

                # rope eviction for q and k at once (8 blocks of 64 ch, each
                # [evens(32) | odds(32)]): A' = A*cos - B*sin ;
                # B' = B*cos + A*sin
                qs = qkpool.tile([128, 512], F32, tag="qs", name="qs")
                cos_t = cos_sb[:, tt, :].rearrange("p (g f) -> p g f", f=32)
                sin_t = sin_sb[:, tt, :].rearrange("p (g f) -> p g f", f=32)
                base = qk_ps[:].rearrange("p (g d) -> p g d", d=D)
                dst = qs[:].rearrange("p (g d) -> p g d", d=D)
                A, Bo = base[:, :, 0:32], base[:, :, 32:64]
                Ad, Bd = dst[:, :, 0:32], dst[:, :, 32:64]
                tmp = qkpool.tile([128, 2, 8, 32], F32, tag="ropetmp", name="tmp")
                nc.vector.tensor_mul(Ad, A, cos_t)
                nc.vector.tensor_mul(tmp[:, 0], Bo, sin_t)
                nc.vector.tensor_sub(Ad, Ad, tmp[:, 0])
                nc.vector.tensor_mul(Bd, Bo, cos_t)
                nc.vector.tensor_mul(tmp[:, 1], A, sin_t)
                nc.vector.tensor_add(Bd, Bd, tmp[:, 1])

                # v eviction (strided dest leaves a ones column per head)
                nc.vector.tensor_copy(
                    v_sb[:, tt, :].rearrange("p (h e) -> p h e", e=D + 1)[
                        :, :, 0:D
                    ],
                    v_ps[:].rearrange("p (h d) -> p h d", d=D),
                )
                nc.vector.tensor_copy(
                    v_sb[:, tt, :].rearrange("p (h e) -> p h e", e=D + 1)[
                        :, :, D : D + 1
                    ],
                    onesr_sb[:, 0:4].rearrange("p (h e) -> p h e", e=1),
                )

                # transposes lag one iteration so the in-order PE is not
                # stalled on this tile's DVE rope output
                qs_pend.append((tt, qs))
                lag = 1 if tt < 4 else 2
                while len(qs_pend) > lag:
                    emit_transposes(qs_pend.pop(0))

            def emit_proj(tt):
                for nn in range(2):
                    op = ps1.tile([128, 512], F32, tag="ps1", name="op")
                    for j in range(2):
                        nc.tensor.matmul(
                            op[:],
                            lhsT=y_sb[j][:, tt * 128 : (tt + 1) * 128],
                            rhs=wp_sb[:, j, nn * 512 : (nn + 1) * 512],
                            start=(j == 0),
                            stop=(j == 1),
                        )
                    ob = opool.tile([128, 512], F32, tag="ob", name="ob")
                    nc.vector.tensor_copy(ob[:], op[:])
                    nc.sync.dma_start(
                        out[tt * 128 : (tt + 1) * 128, nn * 512 : (nn + 1) * 512],
                        ob[:],
                    )

            def emit_attention(m, fillers=()):
                # q-chunk m covers q in [256m, 256m+256); k chunks i<=2m+1.
                # Heads are paired per kT/qT tile jj: scores for both heads
                # land in one psum bank so exp/mask run as one wide ACT/DVE
                # op; head interleaving keeps the in-order PE off ACT's back.
                yps = [
                    psy.tile([128, 256], F32, tag="psy", name=f"yp{m}_{h}")
                    for h in range(HPC)
                ]
                def emit_av(pend_av):
                    pi, pn0e, ppTs = pend_av
                    for h in range(HPC):
                        nc.tensor.matmul(
                            yps[h][0:65, pn0e:256],
                            lhsT=v_sb[:, pi, h * (D + 1) : (h + 1) * (D + 1)],
                            rhs=ppTs[h][:, pn0e:256],
                            start=(pi == 0),
                            stop=(pi == 2 * m + 1),
                        )

                pend_av = None
                for i in range(2 * m + 2):
                    n0e = 128 if i == 2 * m + 1 else 0  # valid-col start
                    sps, pTs = [], []
                    for h in range(HPC):
                        jj, hh = h // 2, h % 2
                        sp = ps1.tile([128, 256], F32, tag="ps1", name=f"sp{h}")
                        sps.append(sp)
                        nc.tensor.matmul(
                            sp[:],
                            lhsT=qkT_sb[64 * hh : 64 * hh + 64, 2 + jj, i, :],
                            rhs=qkT_sb[64 * hh : 64 * hh + 64, jj,
                                       2 * m : 2 * m + 2, :]
                            .rearrange("p a b -> p (a b)"),
                            start=True,
                            stop=True,
                        )
                    for h in range(HPC):
                        sp = sps[h]
                        if i >= 2 * m:  # band tile: mask the diagonal block
                            nc.vector.tensor_add(
                                sp[:, n0e : n0e + 128],
                                sp[:, n0e : n0e + 128],
                                tri_sb[:, 0:128],
                            )
                        pT = ppool.tile([128, 256], F32R, tag="pT", name=f"pT{h}")
                        pTs.append(pT)
                        nc.scalar.activation(
                            pT[:, n0e:256],
                            sp[:, n0e:256],
                            mybir.ActivationFunctionType.Exp,
                            scale=1.0 / math.sqrt(D),
                        )
                    if pend_av is not None:
                        emit_av(pend_av)
                        if i % 2 == 0 and fillers:
                            emit_proj(fillers.pop(0))
                    pend_av = (i, n0e, pTs)
                emit_av(pend_av)
                # normalize: y[d, q] * (1 / y[64, q]); batched per op type
                # so the PE's broadcast matmuls queue behind all reciprocals
                recs, rbs_t = [], []
                for h in range(HPC):
                    rec = npool.tile([1, 256], F32, tag="rec", name=f"rec{h}")
                    recs.append(rec)
                    nc.vector.reciprocal(rec[:], yps[h][64:65, :])
                for h in range(HPC):
                    rb = ps1.tile([64, 256], F32, tag="ps1", name=f"rb{h}")
                    nc.tensor.matmul(
                        rb[:], lhsT=onesf_sb[:], rhs=recs[h][:],
                        start=True, stop=True,
                    )
                    rbs = npool.tile([64, 256], F32, tag="rbs", name=f"rbs{h}")
                    nc.scalar.copy(rbs[:], rb[:])
                    rbs_t.append(rbs)
                for h in range(HPC):
                    jj, po = h // 2, 64 * (h % 2)
                    nc.vector.tensor_mul(
                        y_sb[jj][po : po + 64, m * 256 : (m + 1) * 256],
                        yps[h][0:64, :],
                        rbs_t[h][:],
                    )

            # interleaved emission: attention chunk m fires as soon as
            # its q/k tiles are transposed (after phase1(2m+3) with lag-2
            # transposes); proj follows its chunk immediately
            next_m = 0

            pending_proj = []

            def drain_ready_attention(last_transposed):
                nonlocal next_m
                while next_m < 8 and 2 * next_m + 1 <= last_transposed:
                    emit_attention(next_m, pending_proj)
                    pending_proj.extend([2 * next_m, 2 * next_m + 1])
                    next_m += 1

            for tt in range(TT):
                emit_phase1(tt)
                if tt % 2 == 1 and tt < 15:
                    load_cs_chunk(tt // 2 + 1)
                if tt == 3:
                    nc.sync.dma_start(
                        wp_sb[:], wprojT.rearrange("(j p) n -> p j n", p=128)
                    )
                drain_ready_attention(tt - len(qs_pend))
            for pend in qs_pend:
                emit_transposes(pend)
            qs_pend.clear()
            drain_ready_attention(TT - 1)
            for ptt in pending_proj:
                emit_proj(ptt)
            pending_proj.clear()

    _split_sync_waits(nc)
    return nc


_nc_cache = {}


def _get_nc(with_bias=True):
    if with_bias not in _nc_cache:
        _nc_cache[with_bias] = build_nc(with_bias)
    return _nc_cache[with_bias]


_PERM = np.concatenate([np.arange(0, D, 2), np.arange(1, D, 2)])  # [evens|odds]


_consts_cache = None


def _consts():
    """Call-invariant device constants, built once per process."""
    global _consts_cache
    if _consts_cache is None:
        theta = np.exp(
            np.arange(0, D, 2, dtype=np.float64) * (-math.log(ROPE_BASE) / D)
        )
        ang = np.arange(T, dtype=np.float64)[:, None] * theta[None, :]
        cosr = np.ascontiguousarray(np.tile(np.cos(ang), (1, 8)).astype(np.float32))
        sinr = np.ascontiguousarray(np.tile(np.sin(ang), (1, 8)).astype(np.float32))
        kk, qq = np.meshgrid(np.arange(128), np.arange(128), indexing="ij")
        trimask = np.tile(np.where(qq >= kk, 0.0, -1e30), (1, 2)).astype(np.float32)
        ident = np.eye(128, dtype=np.float32)
        onesr = np.ones((128, 128), dtype=np.float32)
        _consts_cache = (cosr, sinr, trimask, ident, onesr)
    return _consts_cache


def make_inputs(x, Wqkv, bqkv, Wproj):
    """Host-side sharding: returns list of 8 per-core input dicts."""
    cosr, sinr, trimask, ident, onesr = _consts()
    xTb = [np.ascontiguousarray(x[b].T) for b in range(B)]

    in_maps = []
    for c in range(N_CORES):
        b, g = divmod(c, 4)
        heads = range(4 * g, 4 * g + 4)
        rows = []
        for part in range(3):  # q, k, v blocks of Wqkv
            for h in heads:
                blk = np.arange(part * C + h * D, part * C + (h + 1) * D)
                rows.append(blk[_PERM] if part < 2 else blk)
        rows = np.concatenate(rows)
        W_s = Wqkv[rows]  # (768, 1024)
        in_maps.append(
            {
                "xT": xTb[b],
                "wqkvT": np.ascontiguousarray(W_s.T),
                "bqkv_s": np.ascontiguousarray(bqkv[rows][None, :]),
                "wprojT": np.ascontiguousarray(
                    Wproj[:, 256 * g : 256 * (g + 1)].T
                ),
                "cosr": cosr,
                "sinr": sinr,
                "trimask": trimask,
                "ident": ident,
                "onesr": onesr,
            }
        )
    return in_maps


def kernel(x, Wqkv, bqkv, Wproj, bproj):
    x = np.asarray(x, dtype=np.float32)
    Wqkv = np.asarray(Wqkv, dtype=np.float32)
    bqkv = np.asarray(bqkv, dtype=np.float32)
    Wproj = np.asarray(Wproj, dtype=np.float32)
    bproj = np.asarray(bproj, dtype=np.float32)

    nc = _get_nc(with_bias=bool(np.any(bqkv)))
    in_maps = make_inputs(x, Wqkv, bqkv, Wproj)
    res = run_bass_kernel_spmd(nc, in_maps, core_ids=list(range(N_CORES)))
    out = np.zeros((B, T, C), dtype=np.float32)
    for c in range(N_CORES):
        out[c // 4] += res.results[c]["out"]
    out += bproj[None, None, :]
    return out

